# revision 59
# baseline (speedup 1.0000x reference)
"""Trainium2 Bass kernel for nn_DDOpGNNUpsample (GNN message passing, cluster graphs).

Structure exploited: edges are exactly all intra-cluster ordered pairs (minus
self loops) of an 8x8 spatial grid per graph (2 graphs, 16384 nodes total).
The per-edge aggregation

    agg_i = (1/cnt_i) * sum_{j in cluster(i), j != i} ||p_i - p_j|| * x_j

collapses to dense per-cluster matmuls against D[j,i] = ||p_i - p_j|| * inv_cnt.
D^2 comes from a rank-4 bf16 Gram: D2[i,j] = a_i . b_j with a = [cx, cy, 1, s],
b = [-2t*cx, -2t*cy, t*s, t] (coords centered per cluster, s = cx^2+cy^2,
t = inv_cnt^2), then D = sqrt(D2 + eps) on the scalar engine, with eps folded
into the activation bias (keeps padding/diagonal entries positive against bf16
cancellation noise; the phantom self-weight sqrt(eps) ~ 1.7e-4 is negligible).

W_rel is folded through the encoder on the host, z = feat^T (W_enc' W_rel)
[n, 8], so the 64-wide encoded x / aggT stages never exist on device:

    out^T = z^T @ D + W_comb^T @ feat,  W_comb = W_enc' W_root + [W_skip; b_rel; 0]

Layout/scheduling:
  - per-slot column widths = max cluster size over cores per size-sorted slot,
    rounded to 4; slots pack greedily into <=512-column PSUM groups, groups
    laid out in descending width so the cheapest group forms the program tail
  - clusters >128 nodes put rows 128..W in 32-row strips vertically packed at
    partition 32*gi (one K-split accumulation into the same out bank)
  - input DMAs split across all three DMA-capable engines (SP / ACT / Pool),
    with early A/B column chunks so group 0's Gram starts ~1.2us sooner and
    the 1283ns Sqrt table load (triggered early by a dummy sqrt) hides under
    the input window
  - one junk matmul during the DMA window starts the PE DVFS ramp
    (0.65 -> 1.2 -> 2.4 GHz over ~3us of continuous busy)
  - out^T accumulates skip+rel per group in PSUM (one accumulation group per
    2KB bank: first matmul starts, last stops), copies PSUM->SBUF on DVE/ACT,
    ships bf16 (upcast on host), per-group DMAs rotating over SP/Pool/ACT

Sharding: 128 clusters -> 16 per NeuronCore (data parallel, weights replicated).
"""
import numpy as np

B, NX, NY = 2, 8, 8
C_IN, HID, C_OUT = 8, 32, 8
ENC = 2 * HID
N_CORES = 8
N_CLUSTERS = B * NX * NY          # 128
NCL = N_CLUSTERS // N_CORES       # 16 clusters per core
EPS = 3e-8                        # sqrt bias (>> bf16 Gram cancellation noise)


def _clusters(coords, batch):
    cx = np.clip((coords[:, 0] * NX).astype(np.int64), 0, NX - 1)
    cy = np.clip((coords[:, 1] * NY).astype(np.int64), 0, NY - 1)
    return batch * (NX * NY) + cx * NY + cy


def _plan(widths):
    """Derive column offsets, groups, and blk1 bookkeeping from slot widths."""
    widths = list(widths)
    coff = np.zeros(NCL + 1, np.int64)
    np.cumsum(widths, out=coff[1:])
    ncol = int(coff[-1])
    # group the size-sorted slots greedily, then lay the groups out in
    # descending total width: the widest group's sqrt/copy/DMA chain runs
    # early and the program tail is the cheapest group's chain
    groups, cur, curw = [], [], 0
    for c in range(NCL):
        if cur and (curw + widths[c] > 512 or len(cur) == 4):
            groups.append(cur)
            cur, curw = [], 0
        cur.append(c)
        curw += widths[c]
    groups.append(cur)
    order = sorted(range(len(groups)),
                   key=lambda g: -sum(widths[c] for c in groups[g]))
    perm = [c for g in order for c in groups[g]]
    widths = [widths[c] for c in perm]
    ngrp, pos = [], 0
    for g in order:
        ngrp.append(list(range(pos, pos + len(groups[g]))))
        pos += len(groups[g])
    groups = ngrp
    coff = np.zeros(NCL + 1, np.int64)
    np.cumsum(widths, out=coff[1:])
    # blk1 strips are vertically packed at rows 32*gi (gi = index within the
    # group); base partitions must stay in {0,32,64} so <=3 blk1 slots per
    # group (guaranteed: 4 widths >128 would exceed the 512-col group cap).
    # d1off[gidx] = (dall1 col offset, zcol ordinal, strip width, blk1 slots)
    d1off, off, ngb = {}, 0, 0
    for gidx, grp in enumerate(groups):
        b = [c for c in grp if widths[c] > 128]
        if b:
            assert len(b) <= 3
            gw1 = max(widths[c] for c in b)
            d1off[gidx] = (off, ngb, gw1, b)
            off += gw1
            ngb += 1
    return widths, coff, ncol, groups, d1off, off, ngb, perm


def _build_bass_program(widths, reps=1):
    import concourse.bass as bass
    import concourse.bacc as bacc
    import concourse.tile as tile
    from concourse import mybir

    f32 = mybir.dt.float32
    bf16 = mybir.dt.bfloat16
    widths, coff, NCOL, groups, d1off, W1TOT, ngb, _perm = _plan(widths)

    nc = bacc.Bacc("TRN2", target_bir_lowering=False)
    # B-side split point: end of the third group (or fewer).  The early
    # chunk rides on ACT so its DMA is short and the activation-table load
    # starts ~1.2us sooner; the rest lands on Pool behind feat, still in
    # time for the later groups' Gram matmuls.
    bsplit = int(coff[groups[min(3, len(groups) - 1)][0]]) if len(groups) > 1 \
        else NCOL
    # A-side split: first two groups arrive fast so group 0's Gram starts
    # ~1.2us earlier; the remainder follows on the same engine
    asplit = int(coff[groups[min(2, len(groups) - 1)][0]]) if len(groups) > 1 \
        else NCOL
    grama1d = nc.dram_tensor("grama1", [4, asplit], bf16, kind="ExternalInput")
    grama2d = nc.dram_tensor("grama2", [4, NCOL - asplit], bf16,
                             kind="ExternalInput")
    gramb1d = nc.dram_tensor("gramb1", [4, bsplit], bf16, kind="ExternalInput")
    gramb2d = nc.dram_tensor("gramb2", [4, NCOL - bsplit], bf16,
                             kind="ExternalInput")
    featd = nc.dram_tensor("feat", [11, NCOL + 16], bf16, kind="ExternalInput")
    out = nc.dram_tensor("out", [C_OUT, NCOL], bf16, kind="ExternalOutput")

    with tile.TileContext(nc) as tc:
        with (
            tc.tile_pool(name="big", bufs=1) as big_pool,
            tc.tile_pool(name="ps_z", bufs=1, space="PSUM") as ps_z,
            tc.tile_pool(name="ps_d", bufs=2, space="PSUM") as ps_d,
            tc.tile_pool(name="ps_d1", bufs=2, space="PSUM") as ps_d1,
            tc.tile_pool(name="ps_o", bufs=3, space="PSUM") as ps_o,
        ):
            # one input DMA per engine so the transfers overlap:
            # A-side on SP, early B chunk on ACT, feat + late B on Pool
            grama1 = big_pool.tile([4, asplit], bf16, tag="grama1")
            nc.sync.dma_start(grama1[:], grama1d[:])
            if NCOL > asplit:
                grama2 = big_pool.tile([4, NCOL - asplit], bf16, tag="grama2")
                nc.sync.dma_start(grama2[:], grama2d[:])
            else:
                grama2 = None
            gramb1 = big_pool.tile([4, bsplit], bf16, tag="gramb1")
            nc.scalar.dma_start(gramb1[:], gramb1d[:])
            feat = big_pool.tile([11, NCOL + 16], bf16, tag="feat")
            nc.gpsimd.dma_start(feat[:], featd[:])
            if NCOL > bsplit:
                gramb2 = big_pool.tile([4, NCOL - bsplit], bf16, tag="gramb2")
                nc.gpsimd.dma_start(gramb2[:], gramb2d[:])
            else:
                gramb2 = None
            def cfa(c0, c1):
                if c1 <= asplit:
                    return grama1[0:4, c0:c1]
                assert c0 >= asplit
                return grama2[0:4, c0 - asplit:c1 - asplit]

            def cfb(c0, c1):
                if c1 <= bsplit:
                    return gramb1[0:4, c0:c1]
                assert c0 >= bsplit
                return gramb2[0:4, c0 - bsplit:c1 - bsplit]

            cf16 = feat[:, 0:NCOL]
            wencrel = feat[:, NCOL:NCOL + 8]
            wcomb = feat[:, NCOL + 8:NCOL + 16]

            for _rep in range(reps):
                _emit_body(nc, big_pool, ps_z, ps_d, ps_d1, ps_o,
                           cfa, cfb, cf16, wencrel, wcomb, out,
                           widths, coff, NCOL, groups, d1off,
                           W1TOT, ngb, mybir, f32, bf16)

    nc.compile()
    # drop activation-table loads for sets no activation in the program
    # uses (the insertion pass emits a spurious set-0 load at block entry
    # that costs 1283ns of ACT time before the input DMA)
    from concourse.hw_specs import get_activation_tables
    table_sets = list(get_activation_tables(nc.m.arch).values())
    universal = set.intersection(*map(set, table_sets))
    used = {i.func for b in nc.m.functions[0].blocks for i in b.instructions
            if isinstance(i, mybir.InstActivation)} - universal
    for blk in nc.m.functions[0].blocks:
        keep = [i for i in blk.instructions
                if not (isinstance(i, mybir.InstLoadActFuncSet)
                        and i.sync_info is None
                        and not (used & table_sets[i.act_func_set_id]))]
        if len(keep) != len(blk.instructions):
            blk.instructions = keep
    return nc


def _emit_body(nc, big_pool, ps_z, ps_d, ps_d1, ps_o,
               cfa, cfb, cf16, wencrel, wcomb, out,
               widths, coff, NCOL, groups, d1off, W1TOT, ngb,
               mybir, f32, bf16):
    Sqrt = mybir.ActivationFunctionType.Sqrt
    ZW = 128 + 8 * ngb

    dall0 = big_pool.tile([128, NCOL], bf16, tag="dall0")
    if W1TOT:
        dall1 = big_pool.tile([128, W1TOT], bf16, tag="dall1", name="dall1")
    else:
        dall1 = None
    z_sb = big_pool.tile([128, ZW], bf16, tag="z_sb")
    osb = big_pool.tile([C_OUT, NCOL], bf16, tag="osb")
    eps_sb = big_pool.tile([128, 1], f32, tag="eps_sb")
    nc.gpsimd.memset(eps_sb[:], EPS)
    # dummy sqrt: hoists the 1283ns Sqrt activation-table load into the
    # input-DMA window instead of serializing it before the first real
    # sqrt (which waits on the first Gram matmul).  Reads one gramb column
    # so it runs right after ACT's own DMA; output is never read.
    scr_sb = big_pool.tile([4, 1], f32, tag="scr_sb")
    nc.scalar.activation(scr_sb[:], cfb(0, 1), Sqrt, bias=eps_sb[0:4, 0:1])

    # PE warm-up: TRN2's tensor engine ramps 0.65 -> 1.2 -> 2.4 GHz with
    # ~3us of continuous busy time; one junk matmul on a memset tile during
    # the input-DMA window starts the ramp early.  Output is never read.
    warm_sb = big_pool.tile([128, 512], bf16, tag="warm_sb")
    nc.vector.memset(warm_sb[:], 0.0)
    w_ps = ps_z.tile([128, 512], f32, tag="z", name="w_ps")
    nc.tensor.matmul(w_ps[:, 0:512], warm_sb[:, 0:128], warm_sb[:, 0:512],
                     start=True, stop=True)

    # PE warm-up: TRN2's tensor engine ramps 0.65 -> 1.2 -> 2.4 GHz with
    # ~3us of continuous busy time.  Junk matmuls on a memset tile during
    # the input-DMA window start the ramp early, so the real matmul stream
    # hits full clock sooner.  Output bank is never read.

    sim_safe = bool(globals().get("SIM_SAFE"))

    # --- z = feat^T @ (W_enc' W_rel): per-slot node-major [n, 8].  blk1
    # remainders pack vertically at rows 32*gi, one 8-col block per group,
    # mirroring the dall1 strip layout so rel1's lhsT/rhs bases match. ---
    z_ps = ps_z.tile([128, 512], f32, tag="z")
    if sim_safe:
        nc.vector.memset(z_ps[:], 0.0)
    for c in range(NCL):
        col, W = int(coff[c]), widths[c]
        R0 = min(W, 128)
        nc.tensor.matmul(z_ps[0:R0, 8 * c:8 * c + 8],
                         cf16[:, col:col + R0], wencrel,
                         start=True, stop=True)
    for gidx in d1off:
        _, gb, _, bslots = d1off[gidx]
        for gi, c in enumerate(bslots):
            col, W = int(coff[c]), widths[c]
            R1 = W - 128
            nc.tensor.matmul(z_ps[32 * gi:32 * gi + R1,
                                  128 + 8 * gb:128 + 8 * gb + 8],
                             cf16[:, col + 128:col + W], wencrel,
                             start=True, stop=True)
    nc.vector.tensor_copy(z_sb[:, 0:128], z_ps[:, 0:128])
    if ngb:
        nc.vector.tensor_copy(z_sb[0:96, 128:ZW], z_ps[0:96, 128:ZW])

    # --- per group: D2 Gram -> sqrt -> out accumulation, software-pipelined
    #     so PE does group g+1's Gram while ACT sqrts group g ---
    d_tiles = {}

    def emit_d(gidx):
        grp = groups[gidx]
        gc0, gw = int(coff[grp[0]]), int(coff[grp[-1] + 1] - coff[grp[0]])
        d0 = ps_d.tile([128, 512], f32, tag="d0")
        if sim_safe:
            nc.vector.memset(d0[:], 0.0)
        for c in grp:
            col, W = int(coff[c]), widths[c]
            gcol = col - gc0
            R0 = min(W, 128)
            nc.tensor.matmul(d0[0:R0, gcol:gcol + W],
                             cfa(col, col + R0), cfb(col, col + W),
                             start=True, stop=True)
        d1 = None
        if gidx in d1off:
            off, gb, gw1, bslots = d1off[gidx]
            d1 = ps_d1.tile([128, 512], f32, tag="d1", name="d1")
            if sim_safe:
                nc.vector.memset(d1[:], 0.0)
            for gi, c in enumerate(bslots):
                col, W = int(coff[c]), widths[c]
                R1 = W - 128
                nc.tensor.matmul(d1[32 * gi:32 * gi + R1, 0:W],
                                 cfa(col + 128, col + W), cfb(col, col + W),
                                 start=True, stop=True)
        # sqrt (ACT), eps in the bias keeps args positive
        nc.scalar.activation(dall0[:, gc0:gc0 + gw], d0[:, 0:gw], Sqrt,
                             bias=eps_sb[0:128, 0:1])
        if d1 is not None:
            off, gb, gw1, bslots = d1off[gidx]
            nr = 32 * len(bslots)
            nc.scalar.activation(dall1[0:nr, off:off + gw1],
                                 d1[0:nr, 0:gw1], Sqrt,
                                 bias=eps_sb[0:nr, 0:1])
        d_tiles[gidx] = d0

    def emit_out(gidx):
        grp = groups[gidx]
        gc0, gw = int(coff[grp[0]]), int(coff[grp[-1] + 1] - coff[grp[0]])
        o_ps = ps_o.tile([C_OUT, 512], f32, tag="o")
        # one accumulation group per PSUM bank: first matmul starts (zeroes
        # the 2KB zero region), only the very last one stops
        mms = [(o_ps[:, 0:gw], wcomb, cf16[:, gc0:gc0 + gw])]
        for c in grp:
            col, W = int(coff[c]), widths[c]
            gcol = col - gc0
            R0 = min(W, 128)
            mms.append((o_ps[:, gcol:gcol + W],
                        z_sb[0:R0, 8 * c:8 * c + 8],
                        dall0[0:R0, col:col + W]))
            if W > 128:
                off, gb, gw1, bslots = d1off[gidx]
                gi = bslots.index(c)
                R1 = W - 128
                mms.append((o_ps[:, gcol:gcol + W],
                            z_sb[32 * gi:32 * gi + R1,
                                 128 + 8 * gb:128 + 8 * gb + 8],
                            dall1[32 * gi:32 * gi + R1, off:off + W]))
        for i, (o, lhs, rhs) in enumerate(mms):
            nc.tensor.matmul(o, lhs, rhs,
                             start=(i == 0), stop=(i == len(mms) - 1))
        # PSUM -> SBUF copies on DVE; the second-to-last goes to ACT (its
        # sqrt chain is finished by then) so the tail copies don't serialize
        ng = len(groups)
        if gidx == ng - 2:
            nc.scalar.copy(osb[:, gc0:gc0 + gw], o_ps[:, 0:gw])
        else:
            nc.vector.tensor_copy(osb[:, gc0:gc0 + gw], o_ps[:, 0:gw])
        # out DMA: alternate SP/Pool, last group on ACT (free after sqrts)
        if gidx == ng - 1:
            eng = nc.scalar
        else:
            eng = nc.sync if gidx % 2 == 0 else nc.gpsimd
        eng.dma_start(out[:, gc0:gc0 + gw], osb[:, gc0:gc0 + gw])

    # software-pipelined emission (dep-tracking order: out(g) after d(g))
    ng = len(groups)
    emit_d(0)
    for g in range(1, ng):
        emit_d(g)
        emit_out(g - 1)
    emit_out(ng - 1)


def _edges_match_cluster_structure(edge_index, sub, sizes):
    """Cheap host check that edge_index == all intra-cluster ordered pairs."""
    E = edge_index.shape[1]
    if E != int((sizes.astype(np.int64) * (sizes.astype(np.int64) - 1)).sum()):
        return False
    src, dst = edge_index[0].astype(np.int64), edge_index[1].astype(np.int64)
    n = sub.shape[0]
    if src.min() < 0 or src.max() >= n or dst.min() < 0 or dst.max() >= n:
        return False
    if not (sub[src] == sub[dst]).all():
        return False
    if (src == dst).any():
        return False
    pairs = src * n + dst
    return np.unique(pairs).size == E


def _reference_fallback(src_node_values, src_coords, src_batch, tgt_node_values,
                        tgt_coords, tgt_batch, edge_index, W_enc, b_enc, W_skip,
                        W_rel, b_rel, W_root):
    pos = np.concatenate([src_coords, tgt_coords], axis=0)
    vals = np.concatenate([src_node_values, tgt_node_values], axis=0)
    x = np.concatenate([vals, pos], axis=1) @ W_enc + b_enc
    N = x.shape[0]
    src_j, dst_i = edge_index[0].astype(np.int64), edge_index[1].astype(np.int64)
    w = np.linalg.norm(pos[src_j] - pos[dst_i], axis=1)
    agg = np.zeros((N, x.shape[1]), np.float32)
    np.add.at(agg, dst_i, w[:, None] * x[src_j])
    cnt = np.zeros(N, np.float32)
    np.add.at(cnt, dst_i, np.ones_like(w, np.float32))
    agg = agg / np.maximum(cnt, 1.0)[:, None]
    out = agg @ W_rel + b_rel + x @ W_root
    return (tgt_node_values @ W_skip + out[src_coords.shape[0]:]).astype(np.float32)


_PROGRAM_CACHE = {}
LAST_RESULT = None
LAST_IN_MAPS = None
LAST_WIDTHS = None


def kernel(**inputs):
    inputs = {k: np.asarray(v) for k, v in inputs.items()}
    src_node_values = inputs["src_node_values"].astype(np.float32, copy=False)
    src_coords = inputs["src_coords"].astype(np.float32, copy=False)
    tgt_node_values = inputs["tgt_node_values"].astype(np.float32, copy=False)
    tgt_coords = inputs["tgt_coords"].astype(np.float32, copy=False)
    W_enc = inputs["W_enc"].astype(np.float32, copy=False)
    b_enc = inputs["b_enc"].astype(np.float32, copy=False)
    W_skip = inputs["W_skip"].astype(np.float32, copy=False)
    W_rel = inputs["W_rel"].astype(np.float32, copy=False)
    b_rel = inputs["b_rel"].astype(np.float32, copy=False)
    W_root = inputs["W_root"].astype(np.float32, copy=False)
    edge_index = inputs["edge_index"]

    pos = np.concatenate([src_coords, tgt_coords], axis=0)
    vals = np.concatenate([src_node_values, tgt_node_values], axis=0)
    batch = np.concatenate([inputs["src_batch"], inputs["tgt_batch"]]).astype(np.int64)
    N = pos.shape[0]
    N_SRC = src_coords.shape[0]

    sub = _clusters(pos, batch)
    sizes = np.bincount(sub, minlength=N_CLUSTERS)
    if len(sizes) != N_CLUSTERS or not _edges_match_cluster_structure(
            edge_index, sub, sizes):
        return _reference_fallback(
            src_node_values, src_coords, inputs["src_batch"], tgt_node_values,
            tgt_coords, inputs["tgt_batch"], edge_index, W_enc, b_enc, W_skip,
            W_rel, b_rel, W_root)

    order = np.argsort(sub, kind="stable")
    starts = np.zeros(N_CLUSTERS + 1, np.int64)
    np.cumsum(sizes, out=starts[1:])
    # per-core slots sorted by size desc; per-slot width = max size over cores
    slot_map = np.zeros((N_CORES, NCL), np.int64)
    for core in range(N_CORES):
        gids = np.arange(core * NCL, (core + 1) * NCL)
        slot_map[core] = gids[np.argsort(-sizes[gids], kind="stable")]
    slot_max = sizes[slot_map].max(axis=0)
    widths = tuple(int(max(8, -(-int(m) // 4) * 4)) for m in slot_max)
    if any(w > 160 for w in widths):
        # >160-node cluster: the 32-row strip scheme does not cover it
        return _reference_fallback(
            src_node_values, src_coords, inputs["src_batch"], tgt_node_values,
            tgt_coords, inputs["tgt_batch"], edge_index, W_enc, b_enc, W_skip,
            W_rel, b_rel, W_root)
    pwidths, coff, NCOL, groups, d1off, W1TOT, ngb, perm = _plan(widths)

    import ml_dtypes
    bf16 = ml_dtypes.bfloat16

    W_enc11 = np.ascontiguousarray(
        np.concatenate([W_enc[0:C_IN], b_enc[None, :], W_enc[C_IN:C_IN + 2]], axis=0))
    skip9 = np.ascontiguousarray(np.concatenate([W_skip, b_rel[None, :]], axis=0))
    W_encrel = W_enc11.astype(np.float64) @ W_rel.astype(np.float64)
    W_comb = W_enc11.astype(np.float64) @ W_root.astype(np.float64)
    W_comb[0:9] += skip9.astype(np.float64)

    bsplit = int(coff[groups[min(3, len(groups) - 1)][0]]) if len(groups) > 1 \
        else NCOL
    asplit = int(coff[groups[min(2, len(groups) - 1)][0]]) if len(groups) > 1 \
        else NCOL
    in_maps = []
    for core in range(N_CORES):
        grama = np.zeros((4, NCOL), np.float32)
        gramb = np.zeros((4, NCOL), np.float32)
        feat = np.zeros((11, NCOL + 16), np.float32)
        for c in range(NCL):
            g = int(slot_map[core][perm[c]])
            n = int(sizes[g])
            idx = order[starts[g]:starts[g + 1]]
            col = int(coff[c])
            if n > 0:
                mx = pos[idx, 0].mean(dtype=np.float64).astype(np.float32)
                my = pos[idx, 1].mean(dtype=np.float64).astype(np.float32)
                cx = pos[idx, 0] - mx
                cy = pos[idx, 1] - my
            else:
                cx = cy = np.zeros(0, np.float32)
            s = (cx * cx + cy * cy).astype(np.float32)
            t = np.float32(1.0 / max(n - 1, 1)) ** 2
            grama[0, col:col + n] = cx
            grama[1, col:col + n] = cy
            grama[2, col:col + n] = 1.0
            grama[3, col:col + n] = s
            gramb[0, col:col + n] = -2.0 * t * cx
            gramb[1, col:col + n] = -2.0 * t * cy
            gramb[2, col:col + n] = t * s
            gramb[3, col:col + n] = t
            feat[0:8, col:col + n] = vals[idx].T
            feat[8, col:col + n] = 1.0
            feat[9, col:col + n] = pos[idx, 0]
            feat[10, col:col + n] = pos[idx, 1]
        feat[:, NCOL:NCOL + 8] = W_encrel.astype(np.float32)
        feat[:, NCOL + 8:NCOL + 16] = W_comb.astype(np.float32)
        in_maps.append({"grama1": np.ascontiguousarray(grama[:, :asplit]).astype(bf16),
                        "grama2": np.ascontiguousarray(grama[:, asplit:]).astype(bf16),
                        "gramb1": np.ascontiguousarray(gramb[:, :bsplit]).astype(bf16),
                        "gramb2": np.ascontiguousarray(gramb[:, bsplit:]).astype(bf16),
                        "feat": feat.astype(bf16)})

    from concourse import bass_utils
    global LAST_IN_MAPS, LAST_WIDTHS
    LAST_IN_MAPS, LAST_WIDTHS = in_maps, widths
    if widths not in _PROGRAM_CACHE:
        _PROGRAM_CACHE[widths] = _build_bass_program(widths)
    nc = _PROGRAM_CACHE[widths]
    import os
    trace = bool(os.environ.get("KERNEL_PROFILE"))
    if trace:
        try:
            from antenv.axon_hooks import get_axon_ntff_profile_hook  # noqa: F401
        except ImportError:
            trace = False
    res = bass_utils.run_bass_kernel_spmd(
        nc, in_maps, core_ids=list(range(N_CORES)), trace=trace)
    global LAST_RESULT
    LAST_RESULT = res
    results = res.results

    out_full = np.zeros((N, C_OUT), np.float32)
    for core in range(N_CORES):
        outT = results[core]["out"]          # [8, NCOL]
        for c in range(NCL):
            g = int(slot_map[core][perm[c]])
            n = int(sizes[g])
            idx = order[starts[g]:starts[g + 1]]
            col = int(coff[c])
            out_full[idx] = outT[:, col:col + n].T
    return out_full[N_SRC:]
